# revision 1
# baseline (speedup 1.0000x reference)
"""Complex LayerNorm TRN2 kernel (nn_Complex_LayerNorm).

Math (per row r over embed dim D, per feature d):
    whiten:  y = C(r) @ (x - mu(r)),  C = inv(sqrtm(cov2x2))
    recolor: z = Wsqrt(d) @ y + bias(d)

Implementation strategy (per core, pure data-parallel over batch):
  rows-on-partitions layout for load + moments (bn_stats / tensor_tensor_reduce),
  then both the whiten-apply and the recolor run on the TensorEngine in fp32r:
    stage1:  psum1 = xr_blk^T @ [diag(i00)|diag(i01)]        (transpose + row-scale)
           + xi_blk^T @ [diag(i01)|diag(i11)]
           + ones_mat @ [diag(-or)|diag(-oi)]                 (mean offsets)
      -> yrT/yiT in feature-major layout
    stage2:  psum2 = yrT_blk @ W1[b] + yiT_blk @ W2[b] + ones_row @ brbi
      where W1/W2 are host-built "interleaved double diagonal" matrices that
      transpose back to row-major AND recolor AND interleave (zr,zi) pairs so
      the final DRAM buffer is directly viewable as complex64.
"""

import numpy as np

import concourse.bacc as bacc
import concourse.tile as tile
from concourse import mybir
from concourse import bass_utils

F32 = mybir.dt.float32
F32R = mybir.dt.float32r
AL = mybir.AluOpType
AF = mybir.ActivationFunctionType

B, S, D = 8, 4096, 1024
R = S               # rows per core (batch dim sharded 1 per core)
NT = R // 128       # 32 row tiles
NB = D // 128       # 8 feature blocks
C1 = 1024.0 / 1023.0  # unbiased variance correction (torch.var ddof=1)


def _build_nc(nt=NT, no_k1=False, no_mm3=False, diag_engine="any", yt_engine="scalar"):
    rows = nt * 128
    nc = bacc.Bacc("TRN2")

    xr_d = nc.dram_tensor("x_real", (rows, D), F32R, kind="ExternalInput").ap()
    xi_d = nc.dram_tensor("x_imag", (rows, D), F32R, kind="ExternalInput").ap()
    w1_d = nc.dram_tensor("w1c", (NB, 128, 256), F32R, kind="ExternalInput").ap()
    w2_d = nc.dram_tensor("w2c", (NB, 128, 256), F32R, kind="ExternalInput").ap()
    bb_d = nc.dram_tensor("brbi", (1, 2 * D), F32R, kind="ExternalInput").ap()
    id_d = nc.dram_tensor("ident", (128, 128), F32, kind="ExternalInput").ap()
    nid_d = nc.dram_tensor("nident", (128, 128), F32, kind="ExternalInput").ap()
    ones_d = nc.dram_tensor("onesm", (128, 128), F32R, kind="ExternalInput").ap()
    out_d = nc.dram_tensor("out", (rows, 2 * D), F32, kind="ExternalOutput").ap()

    with tile.TileContext(nc) as tc:
        with (
            tc.tile_pool(name="const", bufs=1) as pc,
            tc.tile_pool(name="xin", bufs=3) as px,
            tc.tile_pool(name="scratch", bufs=2) as psc,
            tc.tile_pool(name="stats", bufs=3) as pst,
            tc.tile_pool(name="diag", bufs=2) as pdg,
            tc.tile_pool(name="yt", bufs=3) as pyt,
            tc.tile_pool(name="outp", bufs=2) as pout,
            tc.tile_pool(name="ps1", bufs=2, space="PSUM") as ps1,
            tc.tile_pool(name="ps2", bufs=2, space="PSUM") as ps2,
        ):
            # ---- constants ----
            w1c = pc.tile([128, NB, 256], F32R)
            nc.sync.dma_start(out=w1c, in_=w1_d.rearrange("b p n -> p b n"))
            w2c = pc.tile([128, NB, 256], F32R)
            nc.sync.dma_start(out=w2c, in_=w2_d.rearrange("b p n -> p b n"))
            brbi = pc.tile([1, 2 * D], F32R)
            nc.sync.dma_start(out=brbi, in_=bb_d)
            ident = pc.tile([128, 128], F32)
            nc.sync.dma_start(out=ident, in_=id_d)
            nident = pc.tile([128, 128], F32)
            nc.sync.dma_start(out=nident, in_=nid_d)
            onesm = pc.tile([128, 128], F32R)
            nc.sync.dma_start(out=onesm, in_=ones_d)

            for it in range(nt):
                r0 = it * 128
                xr = px.tile([128, D], F32R, tag="xr")
                nc.sync.dma_start(out=xr, in_=xr_d[r0 : r0 + 128, :])
                xi = px.tile([128, D], F32R, tag="xi")
                nc.sync.dma_start(out=xi, in_=xi_d[r0 : r0 + 128, :])
                xrf = xr.bitcast(F32)
                xif = xi.bitcast(F32)

                # ---- moments ----
                ST = pst.tile([128, 26], F32, tag="st")
                bsr = pst.tile([128, 2, 6], F32, tag="bsr")
                nc.vector.bn_stats(out=bsr[:, 0, :], in_=xrf[:, 0:512])
                nc.vector.bn_stats(out=bsr[:, 1, :], in_=xrf[:, 512:1024])
                nc.vector.bn_aggr(out=ST[:, 0:2], in_=bsr)  # mu_r, var_r(biased)
                bsi = pst.tile([128, 2, 6], F32, tag="bsi")
                nc.vector.bn_stats(out=bsi[:, 0, :], in_=xif[:, 0:512])
                nc.vector.bn_stats(out=bsi[:, 1, :], in_=xif[:, 512:1024])
                nc.vector.bn_aggr(out=ST[:, 2:4], in_=bsi)  # mu_i, var_i(biased)
                prod = psc.tile([128, D], F32, tag="prod")
                nc.vector.scalar_tensor_tensor(
                    out=prod,
                    in0=xrf,
                    scalar=1.0,
                    in1=xif,
                    op0=AL.mult,
                    op1=AL.mult,
                    accum_out=ST[:, 4:5],  # sum(xr*xi)
                )

                # ---- per-row 2x2 whitening coefficients ----
                ts = nc.any.tensor_scalar
                # m = mu_r*mu_i ; cov = sri/D - m
                ts(out=ST[:, 5:6], in0=ST[:, 0:1], scalar1=ST[:, 2:3], scalar2=None, op0=AL.mult)
                ts(out=ST[:, 6:7], in0=ST[:, 4:5], scalar1=1.0 / D, scalar2=ST[:, 5:6], op0=AL.mult, op1=AL.subtract)
                # det = vr*vi - cov^2   (vr,vi unbiased => *C1^2)
                ts(out=ST[:, 7:8], in0=ST[:, 1:2], scalar1=ST[:, 3:4], scalar2=None, op0=AL.mult)
                ts(out=ST[:, 8:9], in0=ST[:, 6:7], scalar1=ST[:, 6:7], scalar2=None, op0=AL.mult)
                ts(out=ST[:, 25:26], in0=ST[:, 7:8], scalar1=C1 * C1, scalar2=ST[:, 8:9], op0=AL.mult, op1=AL.subtract)
                # s = sqrt(det); t = sqrt((var_r+var_i)*C1 + 2s)
                nc.scalar.activation(out=ST[:, 9:10], in_=ST[:, 25:26], func=AF.Sqrt)
                ts(out=ST[:, 10:11], in0=ST[:, 9:10], scalar1=2.0, scalar2=None, op0=AL.mult)
                ts(out=ST[:, 11:12], in0=ST[:, 1:2], scalar1=ST[:, 3:4], scalar2=None, op0=AL.add)
                nc.scalar.activation(out=ST[:, 12:13], in_=ST[:, 11:12], func=AF.Sqrt, bias=ST[:, 10:11], scale=C1)
                # inv = 1/(t*s) ; ninv = -inv
                ts(out=ST[:, 13:14], in0=ST[:, 12:13], scalar1=ST[:, 9:10], scalar2=None, op0=AL.mult)
                nc.vector.reciprocal(out=ST[:, 14:15], in_=ST[:, 13:14])
                ts(out=ST[:, 15:16], in0=ST[:, 14:15], scalar1=-1.0, scalar2=None, op0=AL.mult)
                # i00 = (vi + s)*inv ; i01 = -cov*inv ; i11 = (vr + s)*inv
                ts(out=ST[:, 23:24], in0=ST[:, 3:4], scalar1=C1, scalar2=ST[:, 9:10], op0=AL.mult, op1=AL.add)
                ts(out=ST[:, 16:17], in0=ST[:, 23:24], scalar1=ST[:, 14:15], scalar2=None, op0=AL.mult)
                ts(out=ST[:, 17:18], in0=ST[:, 6:7], scalar1=ST[:, 15:16], scalar2=None, op0=AL.mult)
                ts(out=ST[:, 24:25], in0=ST[:, 1:2], scalar1=C1, scalar2=ST[:, 9:10], op0=AL.mult, op1=AL.add)
                ts(out=ST[:, 18:19], in0=ST[:, 24:25], scalar1=ST[:, 14:15], scalar2=None, op0=AL.mult)
                # orp = i00*mu_r + i01*mu_i ; oip = i01*mu_r + i11*mu_i  (positive;
                # sign flipped via the negated identity in the diag build)
                ts(out=ST[:, 19:20], in0=ST[:, 0:1], scalar1=ST[:, 16:17], scalar2=None, op0=AL.mult)
                ts(out=ST[:, 20:21], in0=ST[:, 2:3], scalar1=ST[:, 17:18], scalar2=ST[:, 19:20], op0=AL.mult, op1=AL.add)
                ts(out=ST[:, 21:22], in0=ST[:, 0:1], scalar1=ST[:, 17:18], scalar2=None, op0=AL.mult)
                ts(out=ST[:, 22:23], in0=ST[:, 2:3], scalar1=ST[:, 18:19], scalar2=ST[:, 21:22], op0=AL.mult, op1=AL.add)

                # ---- per-row diagonal matrices (f32r) ----
                dts = nc.vector.tensor_scalar if diag_engine == "vector" else ts
                DG = pdg.tile([128, 5, 128], F32R, tag="dg")
                dts(out=DG[:, 0, :], in0=ident, scalar1=ST[:, 16:17], scalar2=None, op0=AL.mult)
                dts(out=DG[:, 1, :], in0=ident, scalar1=ST[:, 17:18], scalar2=None, op0=AL.mult)
                dts(out=DG[:, 2, :], in0=ident, scalar1=ST[:, 18:19], scalar2=None, op0=AL.mult)
                dts(out=DG[:, 3, :], in0=nident, scalar1=ST[:, 20:21], scalar2=None, op0=AL.mult)
                dts(out=DG[:, 4, :], in0=nident, scalar1=ST[:, 22:23], scalar2=None, op0=AL.mult)

                out_sb = pout.tile([128, 2 * D], F32, tag="osb")

                for h in range(2):  # halftiles (512 feats each)
                    p1 = ps1.tile([128, 1024], F32, tag="p1")
                    for k in range(2):  # psum banks
                        for j in range(2):  # feature blocks in bank
                            b = 2 * k + j
                            gb = 4 * h + b
                            first = j == 0
                            last = j == 1
                            xr_blk = xr[:, 128 * gb : 128 * (gb + 1)]
                            xi_blk = xi[:, 128 * gb : 128 * (gb + 1)]
                            o = p1[:, 256 * b : 256 * (b + 1)]
                            nc.tensor.matmul(o, xr_blk, DG[:, 0:2, :], start=first, stop=False)
                            nc.tensor.matmul(o, xi_blk, DG[:, 1:3, :], start=False, stop=(last and no_mm3))
                            if not no_mm3:
                                nc.tensor.matmul(o, onesm, DG[:, 3:5, :], start=False, stop=last)

                    yt = pyt.tile([128, 1024], F32R, tag="yt")
                    if yt_engine == "scalar":
                        nc.scalar.copy(out=yt, in_=p1)
                    else:
                        nc.vector.tensor_copy(out=yt, in_=p1)

                    p2 = ps2.tile([128, 1024], F32, tag="p2")
                    for k in range(2):
                        for j in range(2):
                            b = 2 * k + j
                            gb = 4 * h + b
                            o = p2[:, 256 * b : 256 * (b + 1)]
                            yrT = yt[:, 256 * b : 256 * b + 128]
                            yiT = yt[:, 256 * b + 128 : 256 * b + 256]
                            nc.tensor.matmul(o, yrT, w1c[:, gb, :], start=(j == 0), stop=False)
                            nc.tensor.matmul(o, yiT, w2c[:, gb, :], start=False, stop=(no_k1 and j == 1))
                        if not no_k1:
                            # rank-1 bias over this bank [128, 512]
                            c0 = 1024 * h + 512 * k
                            nc.tensor.matmul(
                                p2[:, 512 * k : 512 * (k + 1)],
                                onesm[0:1, :],
                                brbi[:, c0 : c0 + 512],
                                start=False,
                                stop=True,
                            )

                    nc.scalar.copy(out=out_sb[:, 1024 * h : 1024 * (h + 1)], in_=p2)

                nc.sync.dma_start(out=out_d[r0 : r0 + 128, :], in_=out_sb)

    nc.finalize()
    return nc


_NC = None


def _get_nc():
    global _NC
    if _NC is None:
        _NC = _build_nc()
    return _NC


def _host_consts(weights, bias_real, bias_imag):
    w = weights.astype(np.float64)
    wr = w[:, 0, 0] ** 2
    wi = w[:, 1, 0] ** 2
    sig = 1.0 / (1.0 + np.exp(-w[:, 2, 0]))
    wc = (sig - 0.5) * 2.0 * np.sqrt(wr * wi)
    sw = np.sqrt(wr * wi - wc * wc)
    tw = np.sqrt(wr + wi + 2.0 * sw)
    w00 = ((wr + sw) / tw).astype(np.float32)
    w01 = (wc / tw).astype(np.float32)
    w11 = ((wi + sw) / tw).astype(np.float32)

    jj = np.arange(128)
    W1 = np.zeros((NB, 128, 256), np.float32)
    W2 = np.zeros((NB, 128, 256), np.float32)
    for b in range(NB):
        f = 128 * b + jj
        W1[b, jj, 2 * jj] = w00[f]
        W1[b, jj, 2 * jj + 1] = w01[f]
        W2[b, jj, 2 * jj] = w01[f]
        W2[b, jj, 2 * jj + 1] = w11[f]

    BRBI = np.empty((1, 2 * D), np.float32)
    BRBI[0, 0::2] = bias_real
    BRBI[0, 1::2] = bias_imag

    I = np.eye(128, dtype=np.float32)
    consts = {
        "w1c": W1,
        "w2c": W2,
        "brbi": BRBI,
        "ident": I,
        "nident": -I,
        "onesm": np.ones((128, 128), np.float32),
    }
    return consts


def _run(x_real, x_imag, weights, bias_real, bias_imag, trace=False):
    nc = _get_nc()
    consts = _host_consts(
        np.asarray(weights, np.float32),
        np.asarray(bias_real, np.float32),
        np.asarray(bias_imag, np.float32),
    )
    xr = np.ascontiguousarray(np.asarray(x_real, np.float32))
    xi = np.ascontiguousarray(np.asarray(x_imag, np.float32))
    in_maps = [
        {"x_real": xr[c], "x_imag": xi[c], **consts} for c in range(B)
    ]
    res = bass_utils.run_bass_kernel_spmd(
        nc, in_maps, core_ids=list(range(B)), trace=trace
    )
    out = np.empty((B, S, D), np.complex64)
    for c in range(B):
        out[c] = np.ascontiguousarray(res.results[c]["out"]).view(np.complex64)
    return out, res


def kernel(x_real, x_imag, weights, bias_real, bias_imag):
    out, _ = _run(x_real, x_imag, weights, bias_real, bias_imag, trace=False)
    return out



# revision 9
# speedup vs baseline: 1.1164x; 1.1164x over previous
"""Complex LayerNorm TRN2 kernel (nn_Complex_LayerNorm).

Math (per row r over embed dim D, per feature d):
    whiten:  y = C(r) @ (x - mu(r)),  C = inv(sqrtm(cov2x2))
    recolor: z = Wsqrt(d) @ y + bias(d)

Implementation strategy (per core, pure data-parallel over batch):
  rows-on-partitions layout for load + moments (bn_stats / stt-accum), then
  x is mean-centered IN PLACE on the Pool engine (per-row scalar subtract),
  and the whiten-apply + recolor both run on the TensorEngine in fp32r:
    stage1:  psum1 = xc_r_blk^T @ [diag(i00)|diag(i01)]
           + xc_i_blk^T @ [diag(i01)|diag(i11)]
      -> yrT/yiT in feature-major layout (transpose + per-row 2x2 whiten)
    stage2:  psum2 = yrT_blk @ W1[b] + yiT_blk @ W2[b] + ones_row @ brbi
      where W1/W2 are "interleaved double diagonal" matrices built ON CHIP
      (2 tensor_scalar ops per block from the identity and a tiny [128,NB,3]
      coefficient DMA) that transpose back to row-major AND recolor AND
      interleave (zr,zi) pairs so the final DRAM buffer is directly viewable
      as complex64.
  Input loads stream on the SP HWDGE queue; output stores go out per
  half-tile on the Pool SWDGE queue, issued one iteration late so their
  semaphore waits never head-of-line block the Pool subtracts.
"""

import numpy as np

import concourse.bacc as bacc
import concourse.tile as tile
from concourse import mybir
from concourse import bass_utils

F32 = mybir.dt.float32
F32R = mybir.dt.float32r
AL = mybir.AluOpType
AF = mybir.ActivationFunctionType

B, S, D = 8, 4096, 1024
R = S               # rows per core (batch dim sharded 1 per core)
NT = R // 128       # 32 row tiles
NB = D // 128       # 8 feature blocks
C1 = 1024.0 / 1023.0  # unbiased variance correction (torch.var ddof=1)


def _build_nc(nt=NT, px_bufs=12):
    rows = nt * 128
    nc = bacc.Bacc("TRN2")

    xr_d = nc.dram_tensor("x_real", (rows, D), F32R, kind="ExternalInput").ap()
    xi_d = nc.dram_tensor("x_imag", (rows, D), F32R, kind="ExternalInput").ap()
    wc_d = nc.dram_tensor("wcol", (128, NB, 3), F32, kind="ExternalInput").ap()
    bb_d = nc.dram_tensor("brbi", (1, 2 * D), F32R, kind="ExternalInput").ap()
    id_d = nc.dram_tensor("ident", (128, 128), F32, kind="ExternalInput").ap()
    ones_d = nc.dram_tensor("onesr", (1, 128), F32R, kind="ExternalInput").ap()
    out_d = nc.dram_tensor("out", (rows, 2 * D), F32, kind="ExternalOutput").ap()

    with tile.TileContext(nc) as tc:
        with (
            tc.tile_pool(name="const", bufs=1) as pc,
            tc.tile_pool(name="xin", bufs=px_bufs) as px,
            tc.tile_pool(name="scratch", bufs=2) as psc,
            tc.tile_pool(name="stats", bufs=3) as pst,
            tc.tile_pool(name="diag", bufs=3) as pdg,
            tc.tile_pool(name="yt", bufs=3) as pyt,
            tc.tile_pool(name="outp", bufs=3) as pout,
            tc.tile_pool(name="ps1", bufs=2, space="PSUM") as ps1,
            tc.tile_pool(name="ps2", bufs=2, space="PSUM") as ps2,
        ):
            # ---- constants (all small DMAs; W1/W2 built on chip) ----
            ident = pc.tile([128, 128], F32)
            nc.sync.dma_start(out=ident, in_=id_d)
            wcol = pc.tile([128, NB, 3], F32)
            nc.sync.dma_start(out=wcol, in_=wc_d)
            brbi = pc.tile([1, 2 * D], F32R)
            nc.sync.dma_start(out=brbi, in_=bb_d)
            onesr = pc.tile([1, 128], F32R)
            nc.sync.dma_start(out=onesr, in_=ones_d)

            # interleaved double-diagonal recolor matrices:
            #   W1[f, b, j, 0] = w00[128b+f]*delta(f,j)   W1[.., 1] = w01*delta
            #   W2[f, b, j, 0] = w01[128b+f]*delta(f,j)   W2[.., 1] = w11*delta
            w1c = pc.tile([128, NB, 128, 2], F32R)
            w2c = pc.tile([128, NB, 128, 2], F32R)
            for b in range(NB):
                for wt, c0, c1 in ((w1c, 0, 1), (w2c, 1, 2)):
                    nc.any.tensor_scalar(
                        out=wt[:, b, :, 0], in0=ident,
                        scalar1=wcol[:, b, c0 : c0 + 1], scalar2=None, op0=AL.mult,
                    )
                    nc.any.tensor_scalar(
                        out=wt[:, b, :, 1], in0=ident,
                        scalar1=wcol[:, b, c1 : c1 + 1], scalar2=None, op0=AL.mult,
                    )

            pending_out = []  # lag-issued output stores (Pool SWDGE queue)

            for it in range(nt):
                # issue the previous tile's output halves first: their waits
                # are satisfied by now, so they don't stall Pool's sequencer
                for args in pending_out:
                    nc.gpsimd.dma_start(**args)
                pending_out = []

                r0 = it * 128
                xr = px.tile([128, D], F32R, tag="xr")
                nc.sync.dma_start(out=xr, in_=xr_d[r0 : r0 + 128, :])
                xi = px.tile([128, D], F32R, tag="xi")
                nc.sync.dma_start(out=xi, in_=xi_d[r0 : r0 + 128, :])
                xrf = xr.bitcast(F32)
                xif = xi.bitcast(F32)

                # ---- moments ----
                ST = pst.tile([128, 21], F32, tag="st")
                bsr = pst.tile([128, 2, 6], F32, tag="bsr")
                nc.vector.bn_stats(out=bsr[:, 0, :], in_=xrf[:, 0:512])
                nc.vector.bn_stats(out=bsr[:, 1, :], in_=xrf[:, 512:1024])
                nc.vector.bn_aggr(out=ST[:, 0:2], in_=bsr)  # mu_r, var_r(biased)
                bsi = pst.tile([128, 2, 6], F32, tag="bsi")
                nc.vector.bn_stats(out=bsi[:, 0, :], in_=xif[:, 0:512])
                nc.vector.bn_stats(out=bsi[:, 1, :], in_=xif[:, 512:1024])
                nc.vector.bn_aggr(out=ST[:, 2:4], in_=bsi)  # mu_i, var_i(biased)
                prod = psc.tile([128, D], F32, tag="prod")
                nc.vector.scalar_tensor_tensor(
                    out=prod,
                    in0=xrf,
                    scalar=1.0,
                    in1=xif,
                    op0=AL.mult,
                    op1=AL.mult,
                    accum_out=ST[:, 4:5],  # sum(xr*xi) over raw x
                )

                # ---- per-row 2x2 whitening coefficients ----
                ts = nc.any.tensor_scalar
                # m = mu_r*mu_i ; cov = sri/D - m
                ts(out=ST[:, 5:6], in0=ST[:, 0:1], scalar1=ST[:, 2:3], scalar2=None, op0=AL.mult)
                ts(out=ST[:, 6:7], in0=ST[:, 4:5], scalar1=1.0 / D, scalar2=ST[:, 5:6], op0=AL.mult, op1=AL.subtract)
                # det = vr*vi*C1^2 - cov^2
                ts(out=ST[:, 7:8], in0=ST[:, 1:2], scalar1=ST[:, 3:4], scalar2=None, op0=AL.mult)
                ts(out=ST[:, 8:9], in0=ST[:, 6:7], scalar1=ST[:, 6:7], scalar2=None, op0=AL.mult)
                ts(out=ST[:, 9:10], in0=ST[:, 7:8], scalar1=C1 * C1, scalar2=ST[:, 8:9], op0=AL.mult, op1=AL.subtract)
                # s = sqrt(det); t = sqrt((var_r+var_i)*C1 + 2s)
                nc.scalar.activation(out=ST[:, 10:11], in_=ST[:, 9:10], func=AF.Sqrt)
                ts(out=ST[:, 11:12], in0=ST[:, 10:11], scalar1=2.0, scalar2=None, op0=AL.mult)
                ts(out=ST[:, 12:13], in0=ST[:, 1:2], scalar1=ST[:, 3:4], scalar2=None, op0=AL.add)
                nc.scalar.activation(out=ST[:, 13:14], in_=ST[:, 12:13], func=AF.Sqrt, bias=ST[:, 11:12], scale=C1)
                # inv = 1/(t*s)
                ts(out=ST[:, 14:15], in0=ST[:, 13:14], scalar1=ST[:, 10:11], scalar2=None, op0=AL.mult)
                nc.vector.reciprocal(out=ST[:, 15:16], in_=ST[:, 14:15])
                # i00 = (vi*C1 + s)*inv ; i01 = -cov*inv ; i11 = (vr*C1 + s)*inv
                ts(out=ST[:, 19:20], in0=ST[:, 3:4], scalar1=C1, scalar2=ST[:, 10:11], op0=AL.mult, op1=AL.add)
                ts(out=ST[:, 16:17], in0=ST[:, 19:20], scalar1=ST[:, 15:16], scalar2=None, op0=AL.mult)
                ts(out=ST[:, 17:18], in0=ST[:, 6:7], scalar1=ST[:, 15:16], scalar2=-1.0, op0=AL.mult, op1=AL.mult)
                ts(out=ST[:, 20:21], in0=ST[:, 1:2], scalar1=C1, scalar2=ST[:, 10:11], op0=AL.mult, op1=AL.add)
                ts(out=ST[:, 18:19], in0=ST[:, 20:21], scalar1=ST[:, 15:16], scalar2=None, op0=AL.mult)

                # ---- center x in place (Pool; per-row scalar subtract) ----
                nc.gpsimd.tensor_scalar(out=xr, in0=xrf, scalar1=ST[:, 0:1], scalar2=None, op0=AL.subtract)
                nc.gpsimd.tensor_scalar(out=xi, in0=xif, scalar1=ST[:, 2:3], scalar2=None, op0=AL.subtract)

                # ---- per-row diagonal whiten matrices (f32r) ----
                DG = pdg.tile([128, 3, 128], F32R, tag="dg")
                ts(out=DG[:, 0, :], in0=ident, scalar1=ST[:, 16:17], scalar2=None, op0=AL.mult)
                ts(out=DG[:, 1, :], in0=ident, scalar1=ST[:, 17:18], scalar2=None, op0=AL.mult)
                ts(out=DG[:, 2, :], in0=ident, scalar1=ST[:, 18:19], scalar2=None, op0=AL.mult)

                out_sb = pout.tile([128, 2 * D], F32, tag="osb")

                for h in range(2):  # halftiles (512 feats each)
                    p1 = ps1.tile([128, 1024], F32, tag="p1")
                    for k in range(2):  # psum banks
                        for j in range(2):  # feature blocks in bank
                            b = 2 * k + j
                            gb = 4 * h + b
                            xr_blk = xr[:, 128 * gb : 128 * (gb + 1)]
                            xi_blk = xi[:, 128 * gb : 128 * (gb + 1)]
                            o = p1[:, 256 * b : 256 * (b + 1)]
                            nc.tensor.matmul(o, xr_blk, DG[:, 0:2, :], start=(j == 0), stop=False)
                            nc.tensor.matmul(o, xi_blk, DG[:, 1:3, :], start=False, stop=(j == 1))

                    yt = pyt.tile([128, 1024], F32R, tag="yt")
                    nc.scalar.copy(out=yt, in_=p1)

                    p2 = ps2.tile([128, 1024], F32, tag="p2")
                    for k in range(2):
                        for j in range(2):
                            b = 2 * k + j
                            gb = 4 * h + b
                            o = p2[:, 256 * b : 256 * (b + 1)]
                            yrT = yt[:, 256 * b : 256 * b + 128]
                            yiT = yt[:, 256 * b + 128 : 256 * b + 256]
                            nc.tensor.matmul(o, yrT, w1c[:, gb], start=(j == 0), stop=False)
                            nc.tensor.matmul(o, yiT, w2c[:, gb], start=False, stop=False)
                        # rank-1 bias over this bank [128, 512]
                        c0 = 1024 * h + 512 * k
                        nc.tensor.matmul(
                            p2[:, 512 * k : 512 * (k + 1)],
                            onesr,
                            brbi[:, c0 : c0 + 512],
                            start=False,
                            stop=True,
                        )

                    oh = out_sb[:, 1024 * h : 1024 * (h + 1)]
                    nc.scalar.copy(out=oh, in_=p2)
                    pending_out.append(
                        dict(out=out_d[r0 : r0 + 128, 1024 * h : 1024 * (h + 1)], in_=oh)
                    )

            for args in pending_out:
                nc.gpsimd.dma_start(**args)

    nc.finalize()
    return nc


_NC = None


def _get_nc():
    global _NC
    if _NC is None:
        _NC = _build_nc()
    return _NC


def _host_consts(weights, bias_real, bias_imag):
    w = weights.astype(np.float64)
    wr = w[:, 0, 0] ** 2
    wi = w[:, 1, 0] ** 2
    sig = 1.0 / (1.0 + np.exp(-w[:, 2, 0]))
    wc = (sig - 0.5) * 2.0 * np.sqrt(wr * wi)
    sw = np.sqrt(wr * wi - wc * wc)
    tw = np.sqrt(wr + wi + 2.0 * sw)
    w00 = ((wr + sw) / tw).astype(np.float32)
    w01 = (wc / tw).astype(np.float32)
    w11 = ((wi + sw) / tw).astype(np.float32)

    # per-partition recolor coefficients: wcol[j, b, :] = (w00, w01, w11)[128b+j]
    WCOL = np.stack(
        [w00.reshape(NB, 128).T, w01.reshape(NB, 128).T, w11.reshape(NB, 128).T],
        axis=-1,
    ).astype(np.float32)

    BRBI = np.empty((1, 2 * D), np.float32)
    BRBI[0, 0::2] = bias_real
    BRBI[0, 1::2] = bias_imag

    consts = {
        "wcol": np.ascontiguousarray(WCOL),
        "brbi": BRBI,
        "ident": np.eye(128, dtype=np.float32),
        "onesr": np.ones((1, 128), np.float32),
    }
    return consts


def _run(x_real, x_imag, weights, bias_real, bias_imag, trace=False):
    nc = _get_nc()
    consts = _host_consts(
        np.asarray(weights, np.float32),
        np.asarray(bias_real, np.float32),
        np.asarray(bias_imag, np.float32),
    )
    xr = np.ascontiguousarray(np.asarray(x_real, np.float32))
    xi = np.ascontiguousarray(np.asarray(x_imag, np.float32))
    in_maps = [
        {"x_real": xr[c], "x_imag": xi[c], **consts} for c in range(B)
    ]
    res = bass_utils.run_bass_kernel_spmd(
        nc, in_maps, core_ids=list(range(B)), trace=trace
    )
    out = np.empty((B, S, D), np.complex64)
    for c in range(B):
        out[c] = np.ascontiguousarray(res.results[c]["out"]).view(np.complex64)
    return out, res


def kernel(x_real, x_imag, weights, bias_real, bias_imag):
    out, _ = _run(x_real, x_imag, weights, bias_real, bias_imag, trace=False)
    return out


# revision 30
# speedup vs baseline: 1.2047x; 1.0791x over previous
"""Complex LayerNorm TRN2 kernel (nn_Complex_LayerNorm).

Math (per row r over embed dim D, per feature d):
    whiten:  y = C(r) @ (x - mu(r)),  C = inv(sqrtm(cov2x2))
    recolor: z = Wsqrt(d) @ y + bias(d)

Implementation strategy (per core, pure data-parallel over batch):
  rows-on-partitions layout for load + moments (bn_stats / stt-accum), then
  x is mean-centered IN PLACE on the Pool engine (per-row scalar subtract),
  and the whiten-apply + recolor both run on the TensorEngine in fp32r:
    stage1:  psum1 = xc_r_blk^T @ [diag(i00)|diag(i01)]
           + xc_i_blk^T @ [diag(i01)|diag(i11)]
      -> yrT/yiT in feature-major layout (transpose + per-row 2x2 whiten)
    stage2:  psum2 = yrT_blk @ W1[b] + yiT_blk @ W2[b] + ones_row @ brbi
      where W1/W2 are "interleaved double diagonal" matrices built ON CHIP
      (2 tensor_scalar ops per block on Pool from the identity and a tiny
      [128,NB,3] coefficient DMA) that transpose back to row-major AND
      recolor AND interleave (zr,zi) pairs so the final DRAM buffer is
      directly viewable as complex64.
  Input loads stream on the SP HWDGE queue; constants load on the Act
  HWDGE queue (so they never delay the first input); output stores go out
  per half-tile on the Pool SWDGE queue, issued one iteration late so
  their semaphore waits never head-of-line block the Pool subtracts.
"""

import numpy as np

import concourse.bacc as bacc
import concourse.tile as tile
from concourse import mybir
from concourse import bass_utils

F32 = mybir.dt.float32
F32R = mybir.dt.float32r
AL = mybir.AluOpType
AF = mybir.ActivationFunctionType

B, S, D = 8, 4096, 1024
R = S               # rows per core (batch dim sharded 1 per core)
NT = R // 128       # 32 row tiles
NB = D // 128       # 8 feature blocks
C1 = 1024.0 / 1023.0  # unbiased variance correction (torch.var ddof=1)


def _build_nc(nt=NT, px_bufs=12):
    rows = nt * 128
    nc = bacc.Bacc("TRN2")

    xr_d = nc.dram_tensor("x_real", (rows, D), F32R, kind="ExternalInput").ap()
    xi_d = nc.dram_tensor("x_imag", (rows, D), F32R, kind="ExternalInput").ap()
    wc_d = nc.dram_tensor("wcol", (128, NB, 3), F32, kind="ExternalInput").ap()
    bb_d = nc.dram_tensor("brbi", (1, 2 * D), F32R, kind="ExternalInput").ap()
    id_d = nc.dram_tensor("ident", (128, 128), F32, kind="ExternalInput").ap()
    ones_d = nc.dram_tensor("onesr", (1, 128), F32R, kind="ExternalInput").ap()
    out_d = nc.dram_tensor("out", (rows, 2 * D), F32, kind="ExternalOutput").ap()

    with tile.TileContext(nc) as tc:
        with (
            tc.tile_pool(name="const", bufs=1) as pc,
            tc.tile_pool(name="xin", bufs=px_bufs) as px,
            tc.tile_pool(name="scratch", bufs=2) as psc,
            tc.tile_pool(name="stats", bufs=3) as pst,
            tc.tile_pool(name="diag", bufs=3) as pdg,
            tc.tile_pool(name="yt", bufs=3) as pyt,
            tc.tile_pool(name="outp", bufs=5) as pout,
            tc.tile_pool(name="ps1", bufs=2, space="PSUM") as ps1,
            tc.tile_pool(name="ps2", bufs=2, space="PSUM") as ps2,
        ):
            # ---- constants (Act queue; W1/W2 built on chip on Pool) ----
            ident = pc.tile([128, 128], F32)
            nc.scalar.dma_start(out=ident, in_=id_d)
            wcol = pc.tile([128, NB, 3], F32)
            nc.scalar.dma_start(out=wcol, in_=wc_d)
            brbi = pc.tile([1, 2 * D], F32R)
            nc.scalar.dma_start(out=brbi, in_=bb_d)
            onesr = pc.tile([1, 128], F32R)
            nc.scalar.dma_start(out=onesr, in_=ones_d)

            # interleaved double-diagonal recolor matrices:
            #   W1[f, b, j, 0] = w00[128b+f]*delta(f,j)   W1[.., 1] = w01*delta
            #   W2[f, b, j, 0] = w01[128b+f]*delta(f,j)   W2[.., 1] = w11*delta
            w1c = pc.tile([128, NB, 128, 2], F32R)
            w2c = pc.tile([128, NB, 128, 2], F32R)
            for b in range(NB):
                for wt, c0, c1 in ((w1c, 0, 1), (w2c, 1, 2)):
                    nc.gpsimd.tensor_scalar(
                        out=wt[:, b, :, 0], in0=ident,
                        scalar1=wcol[:, b, c0 : c0 + 1], scalar2=None, op0=AL.mult,
                    )
                    nc.gpsimd.tensor_scalar(
                        out=wt[:, b, :, 1], in0=ident,
                        scalar1=wcol[:, b, c1 : c1 + 1], scalar2=None, op0=AL.mult,
                    )

            pending_out = []  # lag-issued output stores (Pool SWDGE queue)

            for it in range(nt):
                # issue the previous tile's output halves first: their waits
                # are satisfied by now, so they don't stall the issuing queue
                for args in pending_out:
                    nc.gpsimd.dma_start(**args)
                pending_out = []

                r0 = it * 128
                xr = px.tile([128, D], F32R, tag="xr")
                nc.sync.dma_start(out=xr, in_=xr_d[r0 : r0 + 128, :])
                xi = px.tile([128, D], F32R, tag="xi")
                nc.sync.dma_start(out=xi, in_=xi_d[r0 : r0 + 128, :])
                xrf = xr.bitcast(F32)
                xif = xi.bitcast(F32)

                # ---- moments ----
                ST = pst.tile([128, 21], F32, tag="st")
                bsr = pst.tile([128, 2, 6], F32, tag="bsr")
                nc.vector.bn_stats(out=bsr[:, 0, :], in_=xrf[:, 0:512])
                nc.vector.bn_stats(out=bsr[:, 1, :], in_=xrf[:, 512:1024])
                nc.vector.bn_aggr(out=ST[:, 0:2], in_=bsr)  # mu_r, var_r(biased)
                bsi = pst.tile([128, 2, 6], F32, tag="bsi")
                nc.vector.bn_stats(out=bsi[:, 0, :], in_=xif[:, 0:512])
                nc.vector.bn_stats(out=bsi[:, 1, :], in_=xif[:, 512:1024])
                nc.vector.bn_aggr(out=ST[:, 2:4], in_=bsi)  # mu_i, var_i(biased)
                prod = psc.tile([128, D], F32, tag="prod")
                nc.vector.scalar_tensor_tensor(
                    out=prod,
                    in0=xrf,
                    scalar=1.0,
                    in1=xif,
                    op0=AL.mult,
                    op1=AL.mult,
                    accum_out=ST[:, 4:5],  # sum(xr*xi) over raw x
                )

                # ---- per-row 2x2 whitening coefficients ----
                ts = nc.any.tensor_scalar
                # m = mu_r*mu_i ; cov = sri/D - m
                ts(out=ST[:, 5:6], in0=ST[:, 0:1], scalar1=ST[:, 2:3], scalar2=None, op0=AL.mult)
                ts(out=ST[:, 6:7], in0=ST[:, 4:5], scalar1=1.0 / D, scalar2=ST[:, 5:6], op0=AL.mult, op1=AL.subtract)
                # det = vr*vi*C1^2 - cov^2
                ts(out=ST[:, 7:8], in0=ST[:, 1:2], scalar1=ST[:, 3:4], scalar2=None, op0=AL.mult)
                ts(out=ST[:, 8:9], in0=ST[:, 6:7], scalar1=ST[:, 6:7], scalar2=None, op0=AL.mult)
                ts(out=ST[:, 9:10], in0=ST[:, 7:8], scalar1=C1 * C1, scalar2=ST[:, 8:9], op0=AL.mult, op1=AL.subtract)
                # s = sqrt(det); t = sqrt((var_r+var_i)*C1 + 2s)
                nc.scalar.activation(out=ST[:, 10:11], in_=ST[:, 9:10], func=AF.Sqrt)
                ts(out=ST[:, 11:12], in0=ST[:, 10:11], scalar1=2.0, scalar2=None, op0=AL.mult)
                ts(out=ST[:, 12:13], in0=ST[:, 1:2], scalar1=ST[:, 3:4], scalar2=None, op0=AL.add)
                nc.scalar.activation(out=ST[:, 13:14], in_=ST[:, 12:13], func=AF.Sqrt, bias=ST[:, 11:12], scale=C1)
                # inv = 1/(t*s)
                ts(out=ST[:, 14:15], in0=ST[:, 13:14], scalar1=ST[:, 10:11], scalar2=None, op0=AL.mult)
                nc.vector.reciprocal(out=ST[:, 15:16], in_=ST[:, 14:15])
                # i00 = (vi*C1 + s)*inv ; i01 = -cov*inv ; i11 = (vr*C1 + s)*inv
                ts(out=ST[:, 19:20], in0=ST[:, 3:4], scalar1=C1, scalar2=ST[:, 10:11], op0=AL.mult, op1=AL.add)
                ts(out=ST[:, 16:17], in0=ST[:, 19:20], scalar1=ST[:, 15:16], scalar2=None, op0=AL.mult)
                ts(out=ST[:, 17:18], in0=ST[:, 6:7], scalar1=ST[:, 15:16], scalar2=-1.0, op0=AL.mult, op1=AL.mult)
                ts(out=ST[:, 20:21], in0=ST[:, 1:2], scalar1=C1, scalar2=ST[:, 10:11], op0=AL.mult, op1=AL.add)
                ts(out=ST[:, 18:19], in0=ST[:, 20:21], scalar1=ST[:, 15:16], scalar2=None, op0=AL.mult)

                # ---- center x in place (Pool; per-row scalar subtract) ----
                nc.gpsimd.tensor_scalar(out=xr, in0=xrf, scalar1=ST[:, 0:1], scalar2=None, op0=AL.subtract)
                nc.vector.tensor_scalar(out=xi, in0=xif, scalar1=ST[:, 2:3], scalar2=None, op0=AL.subtract)

                # ---- per-row diagonal whiten matrices (f32r) ----
                DG = pdg.tile([128, 3, 128], F32R, tag="dg")
                ts(out=DG[:, 0, :], in0=ident, scalar1=ST[:, 16:17], scalar2=None, op0=AL.mult)
                ts(out=DG[:, 1, :], in0=ident, scalar1=ST[:, 17:18], scalar2=None, op0=AL.mult)
                ts(out=DG[:, 2, :], in0=ident, scalar1=ST[:, 18:19], scalar2=None, op0=AL.mult)

                out_sb = pout.tile([128, 2 * D], F32, tag="osb")

                for h in range(2):  # halftiles (512 feats each)
                    p1 = ps1.tile([128, 1024], F32, tag="p1")
                    for k in range(2):  # psum banks
                        for j in range(2):  # feature blocks in bank
                            b = 2 * k + j
                            gb = 4 * h + b
                            xr_blk = xr[:, 128 * gb : 128 * (gb + 1)]
                            xi_blk = xi[:, 128 * gb : 128 * (gb + 1)]
                            o = p1[:, 256 * b : 256 * (b + 1)]
                            nc.tensor.matmul(o, xr_blk, DG[:, 0:2, :], start=(j == 0), stop=False)
                            nc.tensor.matmul(o, xi_blk, DG[:, 1:3, :], start=False, stop=(j == 1))

                    yt = pyt.tile([128, 1024], F32R, tag="yt")
                    nc.scalar.copy(out=yt, in_=p1)

                    p2 = ps2.tile([128, 1024], F32, tag="p2")
                    for k in range(2):
                        for j in range(2):
                            b = 2 * k + j
                            gb = 4 * h + b
                            o = p2[:, 256 * b : 256 * (b + 1)]
                            yrT = yt[:, 256 * b : 256 * b + 128]
                            yiT = yt[:, 256 * b + 128 : 256 * b + 256]
                            nc.tensor.matmul(o, yrT, w1c[:, gb], start=(j == 0), stop=False)
                            nc.tensor.matmul(o, yiT, w2c[:, gb], start=False, stop=False)
                        # rank-1 bias over this bank [128, 512]
                        c0 = 1024 * h + 512 * k
                        nc.tensor.matmul(
                            p2[:, 512 * k : 512 * (k + 1)],
                            onesr,
                            brbi[:, c0 : c0 + 512],
                            start=False,
                            stop=True,
                        )

                    oh = out_sb[:, 1024 * h : 1024 * (h + 1)]
                    nc.scalar.copy(out=oh, in_=p2)
                    pending_out.append(
                        dict(out=out_d[r0 : r0 + 128, 1024 * h : 1024 * (h + 1)], in_=oh)
                    )

            for args in pending_out:
                nc.gpsimd.dma_start(**args)

    nc.finalize()
    return nc


_NC = None


def _get_nc():
    global _NC
    if _NC is None:
        _NC = _build_nc()
    return _NC


def _host_consts(weights, bias_real, bias_imag):
    w = weights.astype(np.float64)
    wr = w[:, 0, 0] ** 2
    wi = w[:, 1, 0] ** 2
    sig = 1.0 / (1.0 + np.exp(-w[:, 2, 0]))
    wc = (sig - 0.5) * 2.0 * np.sqrt(wr * wi)
    sw = np.sqrt(wr * wi - wc * wc)
    tw = np.sqrt(wr + wi + 2.0 * sw)
    w00 = ((wr + sw) / tw).astype(np.float32)
    w01 = (wc / tw).astype(np.float32)
    w11 = ((wi + sw) / tw).astype(np.float32)

    # per-partition recolor coefficients: wcol[j, b, :] = (w00, w01, w11)[128b+j]
    WCOL = np.stack(
        [w00.reshape(NB, 128).T, w01.reshape(NB, 128).T, w11.reshape(NB, 128).T],
        axis=-1,
    ).astype(np.float32)

    BRBI = np.empty((1, 2 * D), np.float32)
    BRBI[0, 0::2] = bias_real
    BRBI[0, 1::2] = bias_imag

    consts = {
        "wcol": np.ascontiguousarray(WCOL),
        "brbi": BRBI,
        "ident": np.eye(128, dtype=np.float32),
        "onesr": np.ones((1, 128), np.float32),
    }
    return consts


def _run(x_real, x_imag, weights, bias_real, bias_imag, trace=False):
    nc = _get_nc()
    consts = _host_consts(
        np.asarray(weights, np.float32),
        np.asarray(bias_real, np.float32),
        np.asarray(bias_imag, np.float32),
    )
    xr = np.ascontiguousarray(np.asarray(x_real, np.float32))
    xi = np.ascontiguousarray(np.asarray(x_imag, np.float32))
    in_maps = [
        {"x_real": xr[c], "x_imag": xi[c], **consts} for c in range(B)
    ]
    res = bass_utils.run_bass_kernel_spmd(
        nc, in_maps, core_ids=list(range(B)), trace=trace
    )
    out = np.empty((B, S, D), np.complex64)
    for c in range(B):
        out[c] = np.ascontiguousarray(res.results[c]["out"]).view(np.complex64)
    return out, res


def kernel(x_real, x_imag, weights, bias_real, bias_imag):
    out, _ = _run(x_real, x_imag, weights, bias_real, bias_imag, trace=False)
    return out
